# revision 29
# baseline (speedup 1.0000x reference)
"""Multi-head attention on 8 Trainium2 NeuronCores (Bass/Tile).

Sharding: core c handles batch b = c//2 and head-half hh = c%2 (heads
8*hh..8*hh+7 = 512 of the 1024 hidden columns). Each core computes its
half-head attention plus the partial output projection through
Wo[:, cols_half]; the host sums the two partials per batch and adds bo.

Device program (per core):
  A) q/k/v projections (PE) with biases folded in (ACT bias on the q/k
     PSUM->SBUF copy, K=1 rank-1 matmul for bv).
  B) attention, two heads ("pair") at a time packed into the 128-wide PE:
     - QK^T row-packed: head A contracts over PE rows 0-63, head B over
       64-127, concurrently (tile_position row groups).
     - masked scores ms = score * mask on DVE straight out of PSUM.
       The reference masks with -1e-12 ~= 0, so exp(mask*score) is exact:
       masked -> exp(0) = 1. No (E-1)*m+1 correction machinery needed.
     - exp on ACT from SBUF in N=4096 ops (amortizes the per-op ramp).
     - AV and the all-ones denominator-broadcast matmuls col-packed
       (head A -> PSUM partitions 0-63, head B -> 64-127) so the
       denominator arrives pre-broadcast across the 64 head dims.
     - normalize: reciprocal_approx_fast on the [128,512] denominator
       PSUM tile, one tensor_tensor multiply -> outnT (bf16).
  C) output projection outnT.T @ woT per 128-row slice, DMA to HBM.
"""

import numpy as np
from contextlib import ExitStack

import ml_dtypes

import concourse.bacc as bacc
import concourse.mybir as mybir
import concourse.tile as tile

F32 = mybir.dt.float32
FP16 = mybir.dt.float16
BF16 = mybir.dt.bfloat16
AF = mybir.ActivationFunctionType
MUL = mybir.AluOpType.mult

B, S, D, H = 4, 2048, 1024, 16
HD = 64
P = 128
JC = 512          # head-cols per core
DC = D // P       # 8 d-chunks
JCH = JC // P     # 4 j-chunks (= head pairs)
SC = S // P       # 16 k-chunks of 128
NQ = 512          # q processed in chunks of 512
NQC = S // NQ     # 4
KG = 2            # k-chunks per exp group (exp op N = KG*2*NQ = 2048)
NKG = SC // KG    # 8
JCA = 8 * (HD + 1)  # 520: v-proj output cols, ones column per head baked in


def build_nc(DT=mybir.dt.float16, n_reps=1, rep_stage="both", cfg=None):
    """One NeuronCore's program: projections + attention in one scope so the
    Tile scheduler overlaps them (no pool-teardown barrier between stages)."""
    cfg = cfg or {}
    nc = bacc.Bacc("TRN2", target_bir_lowering=False, debug=False)

    def mm(out, lhsT, rhs, **kw):
        nc.tensor.matmul(out, lhsT, rhs, **kw)

    xqT = nc.dram_tensor("xqT", [D, S], FP16, kind="ExternalInput").ap()
    xkT = nc.dram_tensor("xkT", [D, S], FP16, kind="ExternalInput").ap()
    xvT = nc.dram_tensor("xvT", [D, S], FP16, kind="ExternalInput").ap()
    wqT = nc.dram_tensor("wqT", [D, JC], FP16, kind="ExternalInput").ap()
    wkT = nc.dram_tensor("wkT", [D, JC], FP16, kind="ExternalInput").ap()
    wvT = nc.dram_tensor("wvT", [D, JCA], FP16, kind="ExternalInput").ap()
    bq = nc.dram_tensor("bq", [JC], F32, kind="ExternalInput").ap()
    bk = nc.dram_tensor("bk", [JC], F32, kind="ExternalInput").ap()
    bv = nc.dram_tensor("bv", [JCA], F32, kind="ExternalInput").ap()
    woT = nc.dram_tensor("woT", [JC, D], BF16, kind="ExternalInput").ap()
    maskT = nc.dram_tensor("maskT", [S, S], F32, kind="ExternalInput").ap()
    outp = nc.dram_tensor("outp", [S, D], F32, kind="ExternalOutput").ap()

    with (
        tile.TileContext(nc) as tc,
        nc.allow_low_precision(reason="fp16/bf16 attention internals"),
        ExitStack() as scope,
    ):
        # ---- long-lived SBUF ----
        pers = scope.enter_context(tc.tile_pool(name="pers", bufs=1))
        # fp16 q/k: halves SBUF vs fp32 and streams the PE at full rate;
        # fp16's 11-bit mantissa keeps score error well under bf16's.
        qT_sb = pers.tile([P, JCH, S], FP16)        # [j%128, pair, q]
        kT_sb = pers.tile([P, JCH, S], FP16)
        v_sb = pers.tile([P, SC, 8, HD + 1], BF16)  # [k%128, kc, head, hd+ones]
        brow = pers.tile([1, P + JCA], F32)         # [ones_c | bv_row]
        bqk = pers.tile([P, 2 * JCH], F32)          # [bq | bk] per-partition
        woT_sb = pers.tile([P, JCH, D], BF16)
        ones_c = brow[:, 0:P]
        bv_row = brow[:, P : P + JCA]
        bq_sb = bqk[:, 0:JCH]
        bk_sb = bqk[:, JCH : 2 * JCH]

        nc.vector.memset(ones_c, 1.0)
        nc.sync.dma_start(bv_row, bv.rearrange("(o j) -> o j", o=1))
        nc.sync.dma_start(bq_sb, bq.rearrange("(c p) -> p c", p=P))
        nc.sync.dma_start(bk_sb, bk.rearrange("(c p) -> p c", p=P))
        nc.sync.dma_start(woT_sb[:], woT.rearrange("(c p) j -> p c j", p=P))

        # ---- shared pools (one scope: proj and attention interleave) ----
        xpool = scope.enter_context(tc.tile_pool(name="xp", bufs=cfg.get("xbufs", 2)))
        wpool = scope.enter_context(tc.tile_pool(name="wp", bufs=cfg.get("wbufs", 2)))
        mpool = scope.enter_context(tc.tile_pool(name="mp", bufs=cfg.get("mbufs", 10)))
        mspool = scope.enter_context(tc.tile_pool(name="ms", bufs=cfg.get("msbufs", 4)))
        epool = scope.enter_context(tc.tile_pool(name="ep", bufs=cfg.get("ebufs", 4)))
        onpool = scope.enter_context(tc.tile_pool(name="on", bufs=cfg.get("onbufs", 2)))
        spool = scope.enter_context(
            tc.tile_pool(name="sp", bufs=cfg.get("sbufs", 3), space="PSUM")
        )
        opool = scope.enter_context(tc.tile_pool(name="op", bufs=cfg.get("obufs", 3), space="PSUM"))
        rpool = scope.enter_context(tc.tile_pool(name="rp", bufs=cfg.get("rbufs", 2)))
        obuf = scope.enter_context(tc.tile_pool(name="ob", bufs=cfg.get("obbufs", 1)))
        if n_reps > 1:
            rep = scope.enter_context(tc.For_i(0, n_reps, 1))

        def proj_qk_block(xT, wT_sb, b_sb, dst, s4):
            """One 512-col block of a q/k projection, 2 j-chunks per PSUM tile."""
            xblk = xpool.tile([P, DC, NQ], FP16, tag="x")
            nc.sync.dma_start(
                xblk[:],
                xT[:, s4 * NQ : (s4 + 1) * NQ].rearrange("(dc p) s -> p dc s", p=P),
            )
            for jj in range(2):
                ps = spool.tile([P, 2, NQ], F32, tag="sc")
                for j2 in range(2):
                    j = 2 * jj + j2
                    for d in range(DC):
                        mm(
                            ps[:, j2, :],
                            wT_sb[:, d, j * P : (j + 1) * P],
                            xblk[:, d, :],
                            start=(d == 0),
                            stop=(d == DC - 1),
                        )
                for j2 in range(2):
                    j = 2 * jj + j2
                    nc.scalar.activation(
                        dst[:, j, s4 * NQ : (s4 + 1) * NQ],
                        ps[:, j2, :],
                        AF.Identity,
                        bias=b_sb[:, j : j + 1],
                    )

        # k projection (whole S) -- attention kc-group g only needs k-block g
        wk_sb = wpool.tile([P, DC, JCA], FP16, tag="w")
        nc.sync.dma_start(wk_sb[:, :, 0:JC], wkT.rearrange("(dc p) j -> p dc j", p=P))
        for s4 in range(NQC):
            proj_qk_block(xkT, wk_sb, bk_sb, kT_sb, s4)

        # v projection (whole S): out chunks [s 128, j 520] (+bv/ones via K=1)
        wv_sb = wpool.tile([P, DC, JCA], FP16, tag="w")
        nc.sync.dma_start(wv_sb[:], wvT.rearrange("(dc p) j -> p dc j", p=P))
        HJ = JCA // 2  # 260
        for s4 in range(NQC):
            xblk = xpool.tile([P, DC, NQ], FP16, tag="x")
            nc.sync.dma_start(
                xblk[:],
                xvT[:, s4 * NQ : (s4 + 1) * NQ].rearrange("(dc p) s -> p dc s", p=P),
            )
            for s16 in range(4):
                sc = s4 * 4 + s16
                # [P, 2, NQ] keeps each 260-col half bank-aligned
                ps = spool.tile([P, 2, NQ], F32, tag="sc")
                for half in range(2):
                    for d in range(DC):
                        mm(
                            ps[:, half, 0:HJ],
                            xblk[:, d, s16 * P : (s16 + 1) * P],
                            wv_sb[:, d, half * HJ : (half + 1) * HJ],
                            start=(d == 0),
                            stop=False,
                        )
                    mm(
                        ps[:, half, 0:HJ],
                        ones_c,
                        bv_row[:, half * HJ : (half + 1) * HJ],
                        start=False,
                        stop=True,
                    )
                nc.scalar.activation(
                    v_sb[:, sc].rearrange("p (a h2) e -> p a (h2 e)", a=2),
                    ps[:, :, 0:HJ],
                    AF.Copy,
                )  # a=2 splits the 8 heads into the two 260-col halves

        # q weights stay resident across the qc loop
        wq_sb = wpool.tile([P, DC, JCA], FP16, tag="w")
        nc.sync.dma_start(wq_sb[:, :, 0:JC], wqT.rearrange("(dc p) j -> p dc j", p=P))

        for qc in range(NQC):
            proj_qk_block(xqT, wq_sb, bq_sb, qT_sb, qc)
            # mask arrives in per-group granules, reused by all 4 head pairs
            mgs = []
            for g in range(NKG):
                mg = mpool.tile([P, KG, NQ], F32, tag="m")
                nc.sync.dma_start(
                    mg[:],
                    maskT[
                        g * KG * P : (g + 1) * KG * P, qc * NQ : (qc + 1) * NQ
                    ].rearrange("(kc p) q -> p kc q", p=P),
                )
                mgs.append(mg)
            outn = onpool.tile([P, JCH, NQ], BF16, tag="on")
            qs = slice(qc * NQ, (qc + 1) * NQ)
            for ch in range(JCH):
                o_ps0 = opool.tile([HD + 1, NQ], F32, tag="ot")
                o_ps1 = opool.tile([HD + 1, NQ], F32, tag="ot")
                o_ps = (o_ps0, o_ps1)

                def emit_av(et, g):
                    for k4 in range(KG):
                        kc = g * KG + k4
                        for hi in range(2):
                            mm(
                                o_ps[hi][:],
                                v_sb[:, kc, 2 * ch + hi, :],
                                et[:, k4, hi, :],
                                start=(kc == 0),
                                stop=(kc == SC - 1),
                            )

                pend = None  # software-pipeline AV one exp-group behind
                for g in range(NKG):
                    ms = mspool.tile([P, KG, 2, NQ], FP16, tag="ms")
                    for k4 in range(KG):
                        kc = g * KG + k4
                        ks = slice(kc * P, (kc + 1) * P)
                        sc_ps = spool.tile([P, 2, NQ], F32, tag="sc")
                        mm(
                            sc_ps[:, 0, :],
                            kT_sb[0:HD, ch, ks],
                            qT_sb[0:HD, ch, qs],
                            start=True,
                            stop=True,
                        )
                        mm(
                            sc_ps[:, 1, :],
                            kT_sb[HD:P, ch, ks],
                            qT_sb[HD:P, ch, qs],
                            start=True,
                            stop=True,
                        )
                        # masked scores straight out of PSUM (DVE); mask
                        # broadcast over the two heads via 0-stride dim
                        nc.vector.tensor_tensor(
                            ms[:, k4, :, :],
                            sc_ps[:],
                            mgs[g][:, k4, :].unsqueeze(1).broadcast_to((P, 2, NQ)),
                            MUL,
                        )
                    et = epool.tile([P, KG, 2, NQ], BF16, tag="e")
                    nc.scalar.activation(et[:], ms[:], AF.Exp)
                    if pend is not None:
                        emit_av(*pend)
                    pend = (et, g)
                emit_av(*pend)
                for hi in range(2):
                    r0 = hi * HD
                    recip = rpool.tile([1, NQ], F32, tag="rc")
                    nc.vector.reciprocal(recip[:], o_ps[hi][HD : HD + 1, :])
                    # rb shares the opool ring (one spare bank at a time)
                    rb_ps = opool.tile([HD + 1, NQ], F32, tag="ot")
                    mm(rb_ps[0:HD, :], ones_c[:, 0:HD], recip[:], start=True, stop=True)
                    rb_sb = rpool.tile([HD, NQ], F32, tag="rb")
                    nc.scalar.activation(rb_sb[:], rb_ps[0:HD, :], AF.Copy)
                    nc.vector.tensor_tensor(
                        outn[r0 : r0 + HD, ch, :],
                        o_ps[hi][0:HD, :],
                        rb_sb[:],
                        MUL,
                    )

            # ---- stage C interleaved: project this qc's finished rows ----
            for sl in range(4):
                sci = 4 * qc + sl
                fps = spool.tile([P, 2, NQ], F32, tag="sc")
                for n2 in range(2):
                    for cc in range(JCH):
                        mm(
                            fps[:, n2, :],
                            outn[:, cc, sl * P : (sl + 1) * P],
                            woT_sb[:, cc, n2 * NQ : (n2 + 1) * NQ],
                            start=(cc == 0),
                            stop=(cc == JCH - 1),
                        )
                ob = obuf.tile([P, D], F32, tag="o")
                nc.scalar.activation(
                    ob[:], fps[:].rearrange("p a b -> p (a b)"), AF.Copy
                )
                nc.sync.dma_start(outp[sci * P : (sci + 1) * P, :], ob[:])

    nc.compile()
    return nc


def _augment_wv(WvJ, f):
    # [512, 1024] row-slice -> transposed + per-head ones column -> [1024, 520]
    out = np.zeros((D, JCA), dtype=f)
    wt = WvJ.T  # [1024, 512]
    for h in range(8):
        out[:, h * (HD + 1) : h * (HD + 1) + HD] = wt[:, h * HD : (h + 1) * HD]
    return out


def _augment_bv(bvJ, f):
    out = np.zeros(JCA, dtype=f)
    for h in range(8):
        out[h * (HD + 1) : h * (HD + 1) + HD] = bvJ[h * HD : (h + 1) * HD]
        out[h * (HD + 1) + HD] = 1.0
    return out


def _prep_in_maps(query, key, value, mask, Wq, bq, Wk, bk, Wv, bv, Wo, bo):
    f = np.float32
    h16 = ml_dtypes.float16 if hasattr(ml_dtypes, "float16") else np.float16
    per_batch = []
    for b in range(B):
        per_batch.append(
            dict(
                xqT=np.ascontiguousarray(query[b].T).astype(h16),
                xkT=np.ascontiguousarray(key[b].T).astype(h16),
                xvT=np.ascontiguousarray(value[b].T).astype(h16),
                maskT=np.ascontiguousarray(mask[b, 0].T).astype(f),
            )
        )
    per_half = []
    for hh in range(2):
        J = slice(JC * hh, JC * (hh + 1))
        per_half.append(
            dict(
                wqT=np.ascontiguousarray(Wq[J].T).astype(h16),
                wkT=np.ascontiguousarray(Wk[J].T).astype(h16),
                wvT=_augment_wv(Wv[J], f).astype(h16),
                bq=np.ascontiguousarray(bq[J], dtype=f),
                bk=np.ascontiguousarray(bk[J], dtype=f),
                bv=_augment_bv(bv[J], f),
                woT=np.ascontiguousarray(Wo[:, J].T).astype(ml_dtypes.bfloat16),
            )
        )
    in_maps = []
    for c in range(8):
        m = dict(per_batch[c // 2])
        m.update(per_half[c % 2])
        in_maps.append(m)
    return in_maps


_NC_CACHE = {}


def _get_nc(dt_name="float32r", n_reps=1, rep_stage="both"):
    key = (dt_name, n_reps, rep_stage)
    if key not in _NC_CACHE:
        _NC_CACHE[key] = build_nc(
            DT=getattr(mybir.dt, dt_name), n_reps=n_reps, rep_stage=rep_stage
        )
    return _NC_CACHE[key]


# ---------------------------------------------------------------------------
# Cached PJRT runner.  Mirrors run_bass_kernel_spmd's axon redirect
# (bass2jax.run_bass_via_pjrt) but builds the jitted shard_map once per
# (dt_name, n_execs) so repeat kernel() calls skip re-tracing, and supports
# chaining n_execs sequential NEFF executions inside one program so test.py
# can measure per-execution hardware time as a slope (dispatch overhead
# cancels).
# ---------------------------------------------------------------------------
_RUNNER_CACHE = {}


def _get_runner(dt_name="float32r", n_reps=1, rep_stage="both"):
    key = (dt_name, n_reps, rep_stage)
    if key in _RUNNER_CACHE:
        return _RUNNER_CACHE[key]

    import jax
    from jax.sharding import Mesh, PartitionSpec
    from jax.experimental.shard_map import shard_map
    from concourse import bass2jax
    from concourse.bass2jax import _bass_exec_p

    bass2jax.install_neuronx_cc_hook()
    nc = _get_nc(dt_name, n_reps, rep_stage)
    partition_name = nc.partition_id_tensor.name if nc.partition_id_tensor else None

    in_names = []
    out_names = []
    out_avals = []
    for alloc in nc.m.functions[0].allocations:
        if not isinstance(alloc, mybir.MemoryLocationSet):
            continue
        name = alloc.memorylocations[0].name
        if alloc.kind == "ExternalInput":
            if name != partition_name:
                in_names.append(name)
        elif alloc.kind == "ExternalOutput":
            out_names.append(name)
            out_avals.append(
                jax.core.ShapedArray(tuple(alloc.tensor_shape), mybir.dt.np(alloc.dtype))
            )
    n_params = len(in_names)
    n_outs = len(out_avals)
    all_in_names = tuple(in_names + out_names)
    if partition_name is not None:
        all_in_names = all_in_names + (partition_name,)

    def _body(*args):
        params = list(args[:n_params])
        zeros = list(args[n_params:])
        pid = (
            [bass2jax.partition_id_tensor()] if partition_name is not None else []
        )
        outs = _bass_exec_p.bind(
            *params,
            *zeros,
            *pid,
            out_avals=tuple(out_avals),
            in_names=all_in_names,
            out_names=tuple(out_names),
            lowering_input_output_aliases=(),
            sim_require_finite=True,
            sim_require_nnan=True,
            nc=nc,
        )
        return tuple(outs)

    devices = jax.devices()[:8]
    mesh = Mesh(np.asarray(devices), ("core",))
    in_specs = (PartitionSpec("core"),) * (n_params + n_outs)
    out_specs = (PartitionSpec("core"),) * n_outs
    fn = jax.jit(
        shard_map(_body, mesh=mesh, in_specs=in_specs, out_specs=out_specs,
                  check_rep=False),
        keep_unused=True,
    )
    runner = (fn, in_names, out_names, out_avals)
    _RUNNER_CACHE[key] = runner
    return runner


def _concat_inputs(in_maps, in_names, out_avals, out_names):
    args = []
    for name in in_names:
        args.append(np.concatenate([np.asarray(m[name]) for m in in_maps], axis=0))
    for i, name in enumerate(out_names):
        z = out_avals[i]
        args.append(np.zeros((8 * z.shape[0], *z.shape[1:]), z.dtype))
    return args


def run(inputs, dt_name="float32r"):
    """Returns (full_output [B,S,D] f32, per-core outp list)."""
    fn, in_names, out_names, out_avals = _get_runner(dt_name, 1)
    in_maps = _prep_in_maps(**inputs)
    args = _concat_inputs(in_maps, in_names, out_avals, out_names)
    out_arrs = fn(*args)
    i = out_names.index("outp")
    per_core = np.asarray(out_arrs[i]).reshape(8, S, D)
    bo = np.asarray(inputs["bo"], dtype=np.float32)
    out = np.empty((B, S, D), dtype=np.float32)
    for b in range(B):
        out[b] = per_core[2 * b] + per_core[2 * b + 1] + bo
    return out, per_core


def bench(inputs, dt_name="float32r", n_reps=1, iters=6, rep_stage="both"):
    """Time the NEFF whose body repeats n_reps times on-device."""
    import time as _time
    import jax
    fn, in_names, out_names, out_avals = _get_runner(dt_name, n_reps, rep_stage)
    in_maps = _prep_in_maps(**inputs)
    args = _concat_inputs(in_maps, in_names, out_avals, out_names)
    dargs = [jax.device_put(a) for a in args]
    times = []
    for _ in range(iters):
        t0 = _time.perf_counter()
        outs = fn(*dargs)
        jax.block_until_ready(outs)
        times.append(_time.perf_counter() - t0)
    return times


def kernel(**inputs):
    inputs = {k: np.asarray(v) for k, v in inputs.items()}
    out, _ = run(inputs)
    return out


# revision 32
# speedup vs baseline: 1.4175x; 1.4175x over previous
"""Multi-head attention on 8 Trainium2 NeuronCores (Bass/Tile).

Sharding: core c handles batch b = c//2 and head-half hh = c%2 (heads
8*hh..8*hh+7 = 512 of the 1024 hidden columns). Each core computes its
half-head attention plus the partial output projection through
Wo[:, cols_half]; the host sums the two partials per batch and adds bo.

Device program (per core):
  A) q/k/v projections (PE) with biases folded in (ACT bias on the q/k
     PSUM->SBUF copy, K=1 rank-1 matmul for bv).
  B) attention, two heads ("pair") at a time packed into the 128-wide PE:
     - QK^T row-packed: head A contracts over PE rows 0-63, head B over
       64-127, concurrently (tile_position row groups).
     - masked scores ms = score * mask on DVE straight out of PSUM.
       The reference masks with -1e-12 ~= 0, so exp(mask*score) is exact:
       masked -> exp(0) = 1. No (E-1)*m+1 correction machinery needed.
     - exp on ACT from SBUF in N=4096 ops (amortizes the per-op ramp).
     - AV and the all-ones denominator-broadcast matmuls col-packed
       (head A -> PSUM partitions 0-63, head B -> 64-127) so the
       denominator arrives pre-broadcast across the 64 head dims.
     - normalize: reciprocal_approx_fast on the [128,512] denominator
       PSUM tile, one tensor_tensor multiply -> outnT (bf16).
  C) output projection outnT.T @ woT per 128-row slice, DMA to HBM.
"""

import numpy as np
from contextlib import ExitStack

import ml_dtypes

import concourse.bacc as bacc
import concourse.mybir as mybir
import concourse.tile as tile

F32 = mybir.dt.float32
FP16 = mybir.dt.float16
BF16 = mybir.dt.bfloat16
AF = mybir.ActivationFunctionType
MUL = mybir.AluOpType.mult

B, S, D, H = 4, 2048, 1024, 16
HD = 64
P = 128
JC = 512          # head-cols per core
DC = D // P       # 8 d-chunks
JCH = JC // P     # 4 j-chunks (= head pairs)
SC = S // P       # 16 k-chunks of 128
NQ = 512          # q processed in chunks of 512
NQC = S // NQ     # 4
KG = 2            # k-chunks per exp group (exp op N = KG*2*NQ = 2048)
NKG = SC // KG    # 8
JCA = 8 * (HD + 1)  # 520: v-proj output cols, ones column per head baked in


def build_nc(DT=mybir.dt.float16, n_reps=1, rep_stage="both", cfg=None):
    """One NeuronCore's program: projections + attention in one scope so the
    Tile scheduler overlaps them (no pool-teardown barrier between stages)."""
    cfg = cfg or {}
    nc = bacc.Bacc("TRN2", target_bir_lowering=False, debug=False)

    def mm(out, lhsT, rhs, **kw):
        nc.tensor.matmul(out, lhsT, rhs, **kw)

    xqT = nc.dram_tensor("xqT", [D, S], FP16, kind="ExternalInput").ap()
    xkT = nc.dram_tensor("xkT", [D, S], FP16, kind="ExternalInput").ap()
    xvT = nc.dram_tensor("xvT", [D, S], FP16, kind="ExternalInput").ap()
    wqT = nc.dram_tensor("wqT", [D, JC], FP16, kind="ExternalInput").ap()
    wkT = nc.dram_tensor("wkT", [D, JC], FP16, kind="ExternalInput").ap()
    wvT = nc.dram_tensor("wvT", [D, JCA], FP16, kind="ExternalInput").ap()
    bq = nc.dram_tensor("bq", [JC], F32, kind="ExternalInput").ap()
    bk = nc.dram_tensor("bk", [JC], F32, kind="ExternalInput").ap()
    bv = nc.dram_tensor("bv", [JCA], F32, kind="ExternalInput").ap()
    woT = nc.dram_tensor("woT", [JC, D], BF16, kind="ExternalInput").ap()
    maskT = nc.dram_tensor("maskT", [S, S], F32, kind="ExternalInput").ap()
    outp = nc.dram_tensor("outp", [S, D], F32, kind="ExternalOutput").ap()

    with (
        tile.TileContext(nc) as tc,
        nc.allow_low_precision(reason="fp16/bf16 attention internals"),
        ExitStack() as scope,
    ):
        # ---- long-lived SBUF ----
        pers = scope.enter_context(tc.tile_pool(name="pers", bufs=1))
        # fp16 q/k: halves SBUF vs fp32 and streams the PE at full rate;
        # fp16's 11-bit mantissa keeps score error well under bf16's.
        qT_sb = pers.tile([P, JCH, S], FP16)        # [j%128, pair, q]
        kT_sb = pers.tile([P, JCH, S], FP16)
        v_sb = pers.tile([P, SC, 8, HD + 1], BF16)  # [k%128, kc, head, hd+ones]
        brow = pers.tile([1, P + JCA], F32)         # [ones_c | bv_row]
        bqk = pers.tile([P, 2 * JCH], F32)          # [bq | bk] per-partition
        woT_sb = pers.tile([P, JCH, D], BF16)
        ones_c = brow[:, 0:P]
        bv_row = brow[:, P : P + JCA]
        bq_sb = bqk[:, 0:JCH]
        bk_sb = bqk[:, JCH : 2 * JCH]

        nc.vector.memset(ones_c, 1.0)
        nc.sync.dma_start(bv_row, bv.rearrange("(o j) -> o j", o=1))
        nc.sync.dma_start(bq_sb, bq.rearrange("(c p) -> p c", p=P))
        nc.sync.dma_start(bk_sb, bk.rearrange("(c p) -> p c", p=P))
        nc.sync.dma_start(woT_sb[:], woT.rearrange("(c p) j -> p c j", p=P))

        # ---- shared pools (one scope: proj and attention interleave) ----
        xpool = scope.enter_context(tc.tile_pool(name="xp", bufs=cfg.get("xbufs", 2)))
        wpool = scope.enter_context(tc.tile_pool(name="wp", bufs=cfg.get("wbufs", 2)))
        mpool = scope.enter_context(tc.tile_pool(name="mp", bufs=cfg.get("mbufs", 10)))
        mspool = scope.enter_context(tc.tile_pool(name="ms", bufs=cfg.get("msbufs", 4)))
        epool = scope.enter_context(tc.tile_pool(name="ep", bufs=cfg.get("ebufs", 4)))
        onpool = scope.enter_context(tc.tile_pool(name="on", bufs=cfg.get("onbufs", 2)))
        spool = scope.enter_context(
            tc.tile_pool(name="sp", bufs=cfg.get("sbufs", 2), space="PSUM")
        )
        opool = scope.enter_context(tc.tile_pool(name="op", bufs=cfg.get("obufs", 3), space="PSUM"))
        rpool = scope.enter_context(tc.tile_pool(name="rp", bufs=cfg.get("rbufs", 4)))
        obuf = scope.enter_context(tc.tile_pool(name="ob", bufs=cfg.get("obbufs", 2)))
        if n_reps > 1:
            rep = scope.enter_context(tc.For_i(0, n_reps, 1))

        def proj_qk_block(xT, wT_sb, b_sb, dst, s4):
            """One 512-col block of a q/k projection, 2 j-chunks per PSUM tile."""
            xblk = xpool.tile([P, DC, NQ], FP16, tag="x")
            nc.sync.dma_start(
                xblk[:],
                xT[:, s4 * NQ : (s4 + 1) * NQ].rearrange("(dc p) s -> p dc s", p=P),
            )
            for jj in range(2):
                ps = spool.tile([P, 2, NQ], F32, tag="sc")
                for j2 in range(2):
                    j = 2 * jj + j2
                    for d in range(DC):
                        mm(
                            ps[:, j2, :],
                            wT_sb[:, d, j * P : (j + 1) * P],
                            xblk[:, d, :],
                            start=(d == 0),
                            stop=(d == DC - 1),
                        )
                for j2 in range(2):
                    j = 2 * jj + j2
                    nc.scalar.activation(
                        dst[:, j, s4 * NQ : (s4 + 1) * NQ],
                        ps[:, j2, :],
                        AF.Identity,
                        bias=b_sb[:, j : j + 1],
                    )

        # k projection (whole S) -- attention kc-group g only needs k-block g
        wk_sb = wpool.tile([P, DC, JCA], FP16, tag="w")
        nc.sync.dma_start(wk_sb[:, :, 0:JC], wkT.rearrange("(dc p) j -> p dc j", p=P))
        for s4 in range(NQC):
            proj_qk_block(xkT, wk_sb, bk_sb, kT_sb, s4)

        # v projection (whole S): out chunks [s 128, j 520] (+bv/ones via K=1)
        wv_sb = wpool.tile([P, DC, JCA], FP16, tag="w")
        nc.sync.dma_start(wv_sb[:], wvT.rearrange("(dc p) j -> p dc j", p=P))
        HJ = JCA // 2  # 260
        for s4 in range(NQC):
            xblk = xpool.tile([P, DC, NQ], FP16, tag="x")
            nc.sync.dma_start(
                xblk[:],
                xvT[:, s4 * NQ : (s4 + 1) * NQ].rearrange("(dc p) s -> p dc s", p=P),
            )
            for s16 in range(4):
                sc = s4 * 4 + s16
                # [P, 2, NQ] keeps each 260-col half bank-aligned
                ps = spool.tile([P, 2, NQ], F32, tag="sc")
                for half in range(2):
                    for d in range(DC):
                        mm(
                            ps[:, half, 0:HJ],
                            xblk[:, d, s16 * P : (s16 + 1) * P],
                            wv_sb[:, d, half * HJ : (half + 1) * HJ],
                            start=(d == 0),
                            stop=False,
                        )
                    mm(
                        ps[:, half, 0:HJ],
                        ones_c,
                        bv_row[:, half * HJ : (half + 1) * HJ],
                        start=False,
                        stop=True,
                    )
                nc.scalar.activation(
                    v_sb[:, sc].rearrange("p (a h2) e -> p a (h2 e)", a=2),
                    ps[:, :, 0:HJ],
                    AF.Copy,
                )  # a=2 splits the 8 heads into the two 260-col halves

        # q weights stay resident across the qc loop
        wq_sb = wpool.tile([P, DC, JCA], FP16, tag="w")
        nc.sync.dma_start(wq_sb[:, :, 0:JC], wqT.rearrange("(dc p) j -> p dc j", p=P))

        for qc in range(NQC):
            proj_qk_block(xqT, wq_sb, bq_sb, qT_sb, qc)
            # mask arrives in per-group granules, reused by all 4 head pairs
            mgs = []
            for g in range(NKG):
                mg = mpool.tile([P, KG, NQ], F32, tag="m")
                nc.sync.dma_start(
                    mg[:],
                    maskT[
                        g * KG * P : (g + 1) * KG * P, qc * NQ : (qc + 1) * NQ
                    ].rearrange("(kc p) q -> p kc q", p=P),
                )
                mgs.append(mg)
            outn = onpool.tile([P, JCH, NQ], BF16, tag="on")
            qs = slice(qc * NQ, (qc + 1) * NQ)
            for ch in range(JCH):
                o_ps0 = opool.tile([HD + 1, NQ], F32, tag="ot")
                o_ps1 = opool.tile([HD + 1, NQ], F32, tag="ot")
                o_ps = (o_ps0, o_ps1)

                def emit_av(et, g):
                    for k4 in range(KG):
                        kc = g * KG + k4
                        for hi in range(2):
                            mm(
                                o_ps[hi][:],
                                v_sb[:, kc, 2 * ch + hi, :],
                                et[:, k4, hi, :],
                                start=(kc == 0),
                                stop=(kc == SC - 1),
                            )

                pend = None  # software-pipeline AV one exp-group behind
                for g in range(NKG):
                    ms = mspool.tile([P, KG, 2, NQ], FP16, tag="ms")
                    for k4 in range(KG):
                        kc = g * KG + k4
                        ks = slice(kc * P, (kc + 1) * P)
                        sc_ps = spool.tile([P, 2, NQ], F32, tag="sc")
                        mm(
                            sc_ps[:, 0, :],
                            kT_sb[0:HD, ch, ks],
                            qT_sb[0:HD, ch, qs],
                            start=True,
                            stop=True,
                        )
                        mm(
                            sc_ps[:, 1, :],
                            kT_sb[HD:P, ch, ks],
                            qT_sb[HD:P, ch, qs],
                            start=True,
                            stop=True,
                        )
                        # masked scores straight out of PSUM (DVE); mask
                        # broadcast over the two heads via 0-stride dim
                        nc.vector.tensor_tensor(
                            ms[:, k4, :, :],
                            sc_ps[:],
                            mgs[g][:, k4, :].unsqueeze(1).broadcast_to((P, 2, NQ)),
                            MUL,
                        )
                    et = epool.tile([P, KG, 2, NQ], BF16, tag="e")
                    nc.scalar.activation(et[:], ms[:], AF.Exp)
                    if pend is not None:
                        emit_av(*pend)
                    pend = (et, g)
                emit_av(*pend)
                for hi in range(2):
                    r0 = hi * HD
                    # denominator row to partition 0 (DVE copy handles the
                    # cross-partition move; recip_approx_fast cannot)
                    dcp = rpool.tile([1, NQ], F32, tag="rc")
                    nc.vector.tensor_copy(dcp[:], o_ps[hi][HD : HD + 1, :])
                    recip = rpool.tile([1, NQ], F32, tag="rc")
                    nc.vector.reciprocal_approx_fast(recip[:], dcp[:])
                    # rb shares the opool ring (one spare bank at a time)
                    rb_ps = opool.tile([HD + 1, NQ], F32, tag="ot")
                    mm(rb_ps[0:HD, :], ones_c[:, 0:HD], recip[:], start=True, stop=True)
                    rb_sb = rpool.tile([HD, NQ], F32, tag="rb")
                    nc.scalar.activation(rb_sb[:], rb_ps[0:HD, :], AF.Copy)
                    nc.vector.tensor_tensor(
                        outn[r0 : r0 + HD, ch, :],
                        o_ps[hi][0:HD, :],
                        rb_sb[:],
                        MUL,
                    )

            # ---- stage C interleaved: project this qc's finished rows ----
            for sl in range(4):
                sci = 4 * qc + sl
                fps = spool.tile([P, 2, NQ], F32, tag="sc")
                for n2 in range(2):
                    for cc in range(JCH):
                        mm(
                            fps[:, n2, :],
                            outn[:, cc, sl * P : (sl + 1) * P],
                            woT_sb[:, cc, n2 * NQ : (n2 + 1) * NQ],
                            start=(cc == 0),
                            stop=(cc == JCH - 1),
                        )
                ob = obuf.tile([P, D], F32, tag="o")
                nc.scalar.activation(
                    ob[:], fps[:].rearrange("p a b -> p (a b)"), AF.Copy
                )
                nc.sync.dma_start(outp[sci * P : (sci + 1) * P, :], ob[:])

    nc.compile()
    return nc


def _augment_wv(WvJ, f):
    # [512, 1024] row-slice -> transposed + per-head ones column -> [1024, 520]
    out = np.zeros((D, JCA), dtype=f)
    wt = WvJ.T  # [1024, 512]
    for h in range(8):
        out[:, h * (HD + 1) : h * (HD + 1) + HD] = wt[:, h * HD : (h + 1) * HD]
    return out


def _augment_bv(bvJ, f):
    out = np.zeros(JCA, dtype=f)
    for h in range(8):
        out[h * (HD + 1) : h * (HD + 1) + HD] = bvJ[h * HD : (h + 1) * HD]
        out[h * (HD + 1) + HD] = 1.0
    return out


def _prep_in_maps(query, key, value, mask, Wq, bq, Wk, bk, Wv, bv, Wo, bo):
    f = np.float32
    h16 = ml_dtypes.float16 if hasattr(ml_dtypes, "float16") else np.float16
    per_batch = []
    for b in range(B):
        per_batch.append(
            dict(
                xqT=np.ascontiguousarray(query[b].T).astype(h16),
                xkT=np.ascontiguousarray(key[b].T).astype(h16),
                xvT=np.ascontiguousarray(value[b].T).astype(h16),
                maskT=np.ascontiguousarray(mask[b, 0].T).astype(f),
            )
        )
    per_half = []
    for hh in range(2):
        J = slice(JC * hh, JC * (hh + 1))
        per_half.append(
            dict(
                wqT=np.ascontiguousarray(Wq[J].T).astype(h16),
                wkT=np.ascontiguousarray(Wk[J].T).astype(h16),
                wvT=_augment_wv(Wv[J], f).astype(h16),
                bq=np.ascontiguousarray(bq[J], dtype=f),
                bk=np.ascontiguousarray(bk[J], dtype=f),
                bv=_augment_bv(bv[J], f),
                woT=np.ascontiguousarray(Wo[:, J].T).astype(ml_dtypes.bfloat16),
            )
        )
    in_maps = []
    for c in range(8):
        m = dict(per_batch[c // 2])
        m.update(per_half[c % 2])
        in_maps.append(m)
    return in_maps


_NC_CACHE = {}


def _get_nc(dt_name="float32r", n_reps=1, rep_stage="both"):
    key = (dt_name, n_reps, rep_stage)
    if key not in _NC_CACHE:
        _NC_CACHE[key] = build_nc(
            DT=getattr(mybir.dt, dt_name), n_reps=n_reps, rep_stage=rep_stage
        )
    return _NC_CACHE[key]


# ---------------------------------------------------------------------------
# Cached PJRT runner.  Mirrors run_bass_kernel_spmd's axon redirect
# (bass2jax.run_bass_via_pjrt) but builds the jitted shard_map once per
# (dt_name, n_execs) so repeat kernel() calls skip re-tracing, and supports
# chaining n_execs sequential NEFF executions inside one program so test.py
# can measure per-execution hardware time as a slope (dispatch overhead
# cancels).
# ---------------------------------------------------------------------------
_RUNNER_CACHE = {}


def _get_runner(dt_name="float32r", n_reps=1, rep_stage="both"):
    key = (dt_name, n_reps, rep_stage)
    if key in _RUNNER_CACHE:
        return _RUNNER_CACHE[key]

    import jax
    from jax.sharding import Mesh, PartitionSpec
    from jax.experimental.shard_map import shard_map
    from concourse import bass2jax
    from concourse.bass2jax import _bass_exec_p

    bass2jax.install_neuronx_cc_hook()
    nc = _get_nc(dt_name, n_reps, rep_stage)
    partition_name = nc.partition_id_tensor.name if nc.partition_id_tensor else None

    in_names = []
    out_names = []
    out_avals = []
    for alloc in nc.m.functions[0].allocations:
        if not isinstance(alloc, mybir.MemoryLocationSet):
            continue
        name = alloc.memorylocations[0].name
        if alloc.kind == "ExternalInput":
            if name != partition_name:
                in_names.append(name)
        elif alloc.kind == "ExternalOutput":
            out_names.append(name)
            out_avals.append(
                jax.core.ShapedArray(tuple(alloc.tensor_shape), mybir.dt.np(alloc.dtype))
            )
    n_params = len(in_names)
    n_outs = len(out_avals)
    all_in_names = tuple(in_names + out_names)
    if partition_name is not None:
        all_in_names = all_in_names + (partition_name,)

    def _body(*args):
        params = list(args[:n_params])
        zeros = list(args[n_params:])
        pid = (
            [bass2jax.partition_id_tensor()] if partition_name is not None else []
        )
        outs = _bass_exec_p.bind(
            *params,
            *zeros,
            *pid,
            out_avals=tuple(out_avals),
            in_names=all_in_names,
            out_names=tuple(out_names),
            lowering_input_output_aliases=(),
            sim_require_finite=True,
            sim_require_nnan=True,
            nc=nc,
        )
        return tuple(outs)

    devices = jax.devices()[:8]
    mesh = Mesh(np.asarray(devices), ("core",))
    in_specs = (PartitionSpec("core"),) * (n_params + n_outs)
    out_specs = (PartitionSpec("core"),) * n_outs
    fn = jax.jit(
        shard_map(_body, mesh=mesh, in_specs=in_specs, out_specs=out_specs,
                  check_rep=False),
        keep_unused=True,
    )
    runner = (fn, in_names, out_names, out_avals)
    _RUNNER_CACHE[key] = runner
    return runner


def _concat_inputs(in_maps, in_names, out_avals, out_names):
    args = []
    for name in in_names:
        args.append(np.concatenate([np.asarray(m[name]) for m in in_maps], axis=0))
    for i, name in enumerate(out_names):
        z = out_avals[i]
        args.append(np.zeros((8 * z.shape[0], *z.shape[1:]), z.dtype))
    return args


def run(inputs, dt_name="float32r"):
    """Returns (full_output [B,S,D] f32, per-core outp list)."""
    fn, in_names, out_names, out_avals = _get_runner(dt_name, 1)
    in_maps = _prep_in_maps(**inputs)
    args = _concat_inputs(in_maps, in_names, out_avals, out_names)
    out_arrs = fn(*args)
    i = out_names.index("outp")
    per_core = np.asarray(out_arrs[i]).reshape(8, S, D)
    bo = np.asarray(inputs["bo"], dtype=np.float32)
    out = np.empty((B, S, D), dtype=np.float32)
    for b in range(B):
        out[b] = per_core[2 * b] + per_core[2 * b + 1] + bo
    return out, per_core


def bench(inputs, dt_name="float32r", n_reps=1, iters=6, rep_stage="both"):
    """Time the NEFF whose body repeats n_reps times on-device."""
    import time as _time
    import jax
    fn, in_names, out_names, out_avals = _get_runner(dt_name, n_reps, rep_stage)
    in_maps = _prep_in_maps(**inputs)
    args = _concat_inputs(in_maps, in_names, out_avals, out_names)
    dargs = [jax.device_put(a) for a in args]
    times = []
    for _ in range(iters):
        t0 = _time.perf_counter()
        outs = fn(*dargs)
        jax.block_until_ready(outs)
        times.append(_time.perf_counter() - t0)
    return times


def kernel(**inputs):
    inputs = {k: np.asarray(v) for k, v in inputs.items()}
    out, _ = run(inputs)
    return out


# revision 38
# speedup vs baseline: 1.4237x; 1.0044x over previous
"""Multi-head attention on 8 Trainium2 NeuronCores (Bass/Tile).

Sharding: core c handles batch b = c//2 and head-half hh = c%2 (heads
8*hh..8*hh+7 = 512 of the 1024 hidden columns). Each core computes its
half-head attention plus the partial output projection through
Wo[:, cols_half]; the host sums the two partials per batch and adds bo.

Device program (per core), all in one Tile scope so projections and
attention interleave:
  A) q/k/v projections (PE, fp16 inputs/weights) with biases folded in
     (ACT bias on the q/k PSUM->SBUF copy, K=1 rank-1 matmul for bv;
     V is augmented with a per-head ones column so the softmax
     denominator accumulates as the 65th row of the AV matmul).
  B) attention, two heads ("pair") at a time:
     - QK^T per head-half (K=64 contraction, N=512).
     - masked scores ms = score * mask on DVE straight out of PSUM.
       The source module masks with -1e-12 ~= 0, so exp(mask*score) is
       exact: masked -> exp(0) = 1; no correction terms needed.
     - exp on ACT from SBUF fp16 in N=2048 ops, one group (KG=2
       k-chunks) behind the mask multiply; AV matmuls trail one exp
       group behind (software pipeline).
     - normalize: copy the denominator row to partition 0 (DVE),
       reciprocal_approx_fast there, PE rank-1 broadcast to 64 rows,
       ACT copy to SBUF, one tensor_tensor multiply -> outn (bf16).
  C) output projection outn.T @ woT per 128-row slice, DMA to HBM.
"""

import numpy as np
from contextlib import ExitStack

import ml_dtypes

import concourse.bacc as bacc
import concourse.mybir as mybir
import concourse.tile as tile

F32 = mybir.dt.float32
FP16 = mybir.dt.float16
BF16 = mybir.dt.bfloat16
AF = mybir.ActivationFunctionType
MUL = mybir.AluOpType.mult

B, S, D, H = 4, 2048, 1024, 16
HD = 64
P = 128
JC = 512          # head-cols per core
DC = D // P       # 8 d-chunks
JCH = JC // P     # 4 j-chunks (= head pairs)
SC = S // P       # 16 k-chunks of 128
NQ = 512          # q processed in chunks of 512
NQC = S // NQ     # 4
KG = 2            # k-chunks per exp group (exp op N = KG*2*NQ = 2048)
NKG = SC // KG    # 8
JCA = 8 * (HD + 1)  # 520: v-proj output cols, ones column per head baked in


def build_nc(DT=mybir.dt.float16, n_reps=1, rep_stage="both", cfg=None):
    """One NeuronCore's program: projections + attention in one scope so the
    Tile scheduler overlaps them (no pool-teardown barrier between stages)."""
    cfg = cfg or {}
    nc = bacc.Bacc("TRN2", target_bir_lowering=False, debug=False)

    def mm(out, lhsT, rhs, **kw):
        nc.tensor.matmul(out, lhsT, rhs, **kw)

    xqT = nc.dram_tensor("xqT", [D, S], FP16, kind="ExternalInput").ap()
    xkT = nc.dram_tensor("xkT", [D, S], FP16, kind="ExternalInput").ap()
    xvT = nc.dram_tensor("xvT", [D, S], FP16, kind="ExternalInput").ap()
    wqT = nc.dram_tensor("wqT", [D, JC], FP16, kind="ExternalInput").ap()
    wkT = nc.dram_tensor("wkT", [D, JC], FP16, kind="ExternalInput").ap()
    wvT = nc.dram_tensor("wvT", [D, JCA], FP16, kind="ExternalInput").ap()
    bq = nc.dram_tensor("bq", [JC], F32, kind="ExternalInput").ap()
    bk = nc.dram_tensor("bk", [JC], F32, kind="ExternalInput").ap()
    bv = nc.dram_tensor("bv", [JCA], F32, kind="ExternalInput").ap()
    woT = nc.dram_tensor("woT", [JC, D], BF16, kind="ExternalInput").ap()
    maskT = nc.dram_tensor("maskT", [S, S], F32, kind="ExternalInput").ap()
    outp = nc.dram_tensor("outp", [S, D], F32, kind="ExternalOutput").ap()

    with (
        tile.TileContext(nc) as tc,
        nc.allow_low_precision(reason="fp16/bf16 attention internals"),
        ExitStack() as scope,
    ):
        # ---- long-lived SBUF ----
        pers = scope.enter_context(tc.tile_pool(name="pers", bufs=1))
        # fp16 q/k: halves SBUF vs fp32 and streams the PE at full rate;
        # fp16's 11-bit mantissa keeps score error well under bf16's.
        qT_sb = pers.tile([P, JCH, S], FP16)        # [j%128, pair, q]
        kT_sb = pers.tile([P, JCH, S], FP16)
        v_sb = pers.tile([P, SC, 8, HD + 1], BF16)  # [k%128, kc, head, hd+ones]
        brow = pers.tile([1, P + JCA], F32)         # [ones_c | bv_row]
        bqk = pers.tile([P, 2 * JCH], F32)          # [bq | bk] per-partition
        woT_sb = pers.tile([P, JCH, D], BF16)
        ones_c = brow[:, 0:P]
        bv_row = brow[:, P : P + JCA]
        bq_sb = bqk[:, 0:JCH]
        bk_sb = bqk[:, JCH : 2 * JCH]

        nc.vector.memset(ones_c, 1.0)
        nc.sync.dma_start(bv_row, bv.rearrange("(o j) -> o j", o=1))
        nc.sync.dma_start(bq_sb, bq.rearrange("(c p) -> p c", p=P))
        nc.sync.dma_start(bk_sb, bk.rearrange("(c p) -> p c", p=P))
        nc.sync.dma_start(woT_sb[:], woT.rearrange("(c p) j -> p c j", p=P))

        # ---- shared pools (one scope: proj and attention interleave) ----
        xpool = scope.enter_context(tc.tile_pool(name="xp", bufs=cfg.get("xbufs", 2)))
        wpool = scope.enter_context(tc.tile_pool(name="wp", bufs=cfg.get("wbufs", 2)))
        mpool = scope.enter_context(tc.tile_pool(name="mp", bufs=cfg.get("mbufs", 10)))
        mspool = scope.enter_context(tc.tile_pool(name="ms", bufs=cfg.get("msbufs", 4)))
        epool = scope.enter_context(tc.tile_pool(name="ep", bufs=cfg.get("ebufs", 4)))
        onpool = scope.enter_context(tc.tile_pool(name="on", bufs=cfg.get("onbufs", 2)))
        spool = scope.enter_context(
            tc.tile_pool(name="sp", bufs=cfg.get("sbufs", 2), space="PSUM")
        )
        opool = scope.enter_context(tc.tile_pool(name="op", bufs=cfg.get("obufs", 3), space="PSUM"))
        rbpool = scope.enter_context(tc.tile_pool(name="rb", bufs=cfg.get("rbpbufs", 1), space="PSUM"))
        rpool = scope.enter_context(tc.tile_pool(name="rp", bufs=cfg.get("rbufs", 4)))
        obuf = scope.enter_context(tc.tile_pool(name="ob", bufs=cfg.get("obbufs", 2)))
        if n_reps > 1:
            rep = scope.enter_context(tc.For_i(0, n_reps, 1))

        def proj_qk_block(xT, wT_sb, b_sb, dst, s4):
            """One 512-col block of a q/k projection, 2 j-chunks per PSUM tile."""
            xblk = xpool.tile([P, DC, NQ], FP16, tag="x")
            nc.sync.dma_start(
                xblk[:],
                xT[:, s4 * NQ : (s4 + 1) * NQ].rearrange("(dc p) s -> p dc s", p=P),
            )
            for jj in range(2):
                ps = spool.tile([P, 2, NQ], F32, tag="sc")
                for j2 in range(2):
                    j = 2 * jj + j2
                    for d in range(DC):
                        mm(
                            ps[:, j2, :],
                            wT_sb[:, d, j * P : (j + 1) * P],
                            xblk[:, d, :],
                            start=(d == 0),
                            stop=(d == DC - 1),
                        )
                for j2 in range(2):
                    j = 2 * jj + j2
                    nc.scalar.activation(
                        dst[:, j, s4 * NQ : (s4 + 1) * NQ],
                        ps[:, j2, :],
                        AF.Identity,
                        bias=b_sb[:, j : j + 1],
                    )

        # k projection (whole S) -- attention kc-group g only needs k-block g
        wk_sb = wpool.tile([P, DC, JCA], FP16, tag="w")
        nc.sync.dma_start(wk_sb[:, :, 0:JC], wkT.rearrange("(dc p) j -> p dc j", p=P))
        for s4 in range(NQC):
            proj_qk_block(xkT, wk_sb, bk_sb, kT_sb, s4)

        # v projection (whole S): out chunks [s 128, j 520] (+bv/ones via K=1)
        wv_sb = wpool.tile([P, DC, JCA], FP16, tag="w")
        nc.sync.dma_start(wv_sb[:], wvT.rearrange("(dc p) j -> p dc j", p=P))
        HJ = JCA // 2  # 260
        for s4 in range(NQC):
            xblk = xpool.tile([P, DC, NQ], FP16, tag="x")
            nc.sync.dma_start(
                xblk[:],
                xvT[:, s4 * NQ : (s4 + 1) * NQ].rearrange("(dc p) s -> p dc s", p=P),
            )
            for s16 in range(4):
                sc = s4 * 4 + s16
                # [P, 2, NQ] keeps each 260-col half bank-aligned
                ps = spool.tile([P, 2, NQ], F32, tag="sc")
                for half in range(2):
                    for d in range(DC):
                        mm(
                            ps[:, half, 0:HJ],
                            xblk[:, d, s16 * P : (s16 + 1) * P],
                            wv_sb[:, d, half * HJ : (half + 1) * HJ],
                            start=(d == 0),
                            stop=False,
                        )
                    mm(
                        ps[:, half, 0:HJ],
                        ones_c,
                        bv_row[:, half * HJ : (half + 1) * HJ],
                        start=False,
                        stop=True,
                    )
                nc.scalar.activation(
                    v_sb[:, sc].rearrange("p (a h2) e -> p a (h2 e)", a=2),
                    ps[:, :, 0:HJ],
                    AF.Copy,
                )  # a=2 splits the 8 heads into the two 260-col halves

        # q weights stay resident across the qc loop
        wq_sb = wpool.tile([P, DC, JCA], FP16, tag="w")
        nc.sync.dma_start(wq_sb[:, :, 0:JC], wqT.rearrange("(dc p) j -> p dc j", p=P))

        prev_oproj = None  # (outn, qc) awaiting output projection
        for qc in range(NQC):
            proj_qk_block(xqT, wq_sb, bq_sb, qT_sb, qc)
            # mask arrives in per-group granules, reused by all 4 head pairs
            mgs = []
            for g in range(NKG):
                mg = mpool.tile([P, KG, NQ], F32, tag="m")
                nc.sync.dma_start(
                    mg[:],
                    maskT[
                        g * KG * P : (g + 1) * KG * P, qc * NQ : (qc + 1) * NQ
                    ].rearrange("(kc p) q -> p kc q", p=P),
                )
                mgs.append(mg)
            outn = onpool.tile([P, JCH, NQ], BF16, tag="on")
            qs = slice(qc * NQ, (qc + 1) * NQ)

            def emit_oproj(src_outn, src_qc, sl):
                """One 128-row slice of the output projection."""
                sci = 4 * src_qc + sl
                fps = spool.tile([P, 2, NQ], F32, tag="sc")
                for n2 in range(2):
                    for cc in range(JCH):
                        mm(
                            fps[:, n2, :],
                            src_outn[:, cc, sl * P : (sl + 1) * P],
                            woT_sb[:, cc, n2 * NQ : (n2 + 1) * NQ],
                            start=(cc == 0),
                            stop=(cc == JCH - 1),
                        )
                ob = obuf.tile([P, D], F32, tag="o")
                nc.scalar.activation(
                    ob[:], fps[:].rearrange("p a b -> p (a b)"), AF.Copy
                )
                nc.sync.dma_start(outp[sci * P : (sci + 1) * P, :], ob[:])

            for ch in range(JCH):
                # previous qc's output projection rides between this qc's
                # pairs so the PE never idles the other engines at qc
                # boundaries
                if prev_oproj is not None:
                    emit_oproj(*prev_oproj, ch)
                o_ps0 = opool.tile([HD + 1, NQ], F32, tag="ot")
                o_ps1 = opool.tile([HD + 1, NQ], F32, tag="ot")
                o_ps = (o_ps0, o_ps1)

                def emit_av(et, g):
                    for k4 in range(KG):
                        kc = g * KG + k4
                        for hi in range(2):
                            mm(
                                o_ps[hi][:],
                                v_sb[:, kc, 2 * ch + hi, :],
                                et[:, k4, hi, :],
                                start=(kc == 0),
                                stop=(kc == SC - 1),
                            )

                pend = None  # software-pipeline AV one exp-group behind
                for g in range(NKG):
                    ms = mspool.tile([P, KG, 2, NQ], FP16, tag="ms")
                    for k4 in range(KG):
                        kc = g * KG + k4
                        ks = slice(kc * P, (kc + 1) * P)
                        sc_ps = spool.tile([P, 2, NQ], F32, tag="sc")
                        mm(
                            sc_ps[:, 0, :],
                            kT_sb[0:HD, ch, ks],
                            qT_sb[0:HD, ch, qs],
                            start=True,
                            stop=True,
                        )
                        mm(
                            sc_ps[:, 1, :],
                            kT_sb[HD:P, ch, ks],
                            qT_sb[HD:P, ch, qs],
                            start=True,
                            stop=True,
                        )
                        # masked scores straight out of PSUM (DVE); mask
                        # broadcast over the two heads via 0-stride dim
                        nc.vector.tensor_tensor(
                            ms[:, k4, :, :],
                            sc_ps[:],
                            mgs[g][:, k4, :].unsqueeze(1).broadcast_to((P, 2, NQ)),
                            MUL,
                        )
                    et = epool.tile([P, KG, 2, NQ], BF16, tag="e")
                    nc.scalar.activation(et[:], ms[:], AF.Exp)
                    if pend is not None:
                        emit_av(*pend)
                    pend = (et, g)
                emit_av(*pend)
                for hi in range(2):
                    r0 = hi * HD
                    # denominator row to partition 0 (DVE copy handles the
                    # cross-partition move; recip_approx_fast cannot)
                    dcp = rpool.tile([1, NQ], F32, tag="rc")
                    nc.vector.tensor_copy(dcp[:], o_ps[hi][HD : HD + 1, :])
                    recip = rpool.tile([1, NQ], F32, tag="rc")
                    nc.vector.reciprocal_approx_fast(recip[:], dcp[:])
                    rb_ps = rbpool.tile([HD, NQ], F32, tag="rbp")
                    mm(rb_ps[:], ones_c[:, 0:HD], recip[:], start=True, stop=True)
                    rb_sb = rpool.tile([HD, NQ], F32, tag="rb")
                    nc.scalar.activation(rb_sb[:], rb_ps[:], AF.Copy)
                    nc.vector.tensor_tensor(
                        outn[r0 : r0 + HD, ch, :],
                        o_ps[hi][0:HD, :],
                        rb_sb[:],
                        MUL,
                    )

            prev_oproj = (outn, qc)

        # drain the last qc's output projection
        for sl in range(4):
            emit_oproj(*prev_oproj, sl)

    nc.compile()
    return nc


def _augment_wv(WvJ, f):
    # [512, 1024] row-slice -> transposed + per-head ones column -> [1024, 520]
    out = np.zeros((D, JCA), dtype=f)
    wt = WvJ.T  # [1024, 512]
    for h in range(8):
        out[:, h * (HD + 1) : h * (HD + 1) + HD] = wt[:, h * HD : (h + 1) * HD]
    return out


def _augment_bv(bvJ, f):
    out = np.zeros(JCA, dtype=f)
    for h in range(8):
        out[h * (HD + 1) : h * (HD + 1) + HD] = bvJ[h * HD : (h + 1) * HD]
        out[h * (HD + 1) + HD] = 1.0
    return out


def _prep_in_maps(query, key, value, mask, Wq, bq, Wk, bk, Wv, bv, Wo, bo):
    f = np.float32
    h16 = ml_dtypes.float16 if hasattr(ml_dtypes, "float16") else np.float16
    per_batch = []
    for b in range(B):
        per_batch.append(
            dict(
                xqT=np.ascontiguousarray(query[b].T).astype(h16),
                xkT=np.ascontiguousarray(key[b].T).astype(h16),
                xvT=np.ascontiguousarray(value[b].T).astype(h16),
                maskT=np.ascontiguousarray(mask[b, 0].T).astype(f),
            )
        )
    per_half = []
    for hh in range(2):
        J = slice(JC * hh, JC * (hh + 1))
        per_half.append(
            dict(
                wqT=np.ascontiguousarray(Wq[J].T).astype(h16),
                wkT=np.ascontiguousarray(Wk[J].T).astype(h16),
                wvT=_augment_wv(Wv[J], f).astype(h16),
                bq=np.ascontiguousarray(bq[J], dtype=f),
                bk=np.ascontiguousarray(bk[J], dtype=f),
                bv=_augment_bv(bv[J], f),
                woT=np.ascontiguousarray(Wo[:, J].T).astype(ml_dtypes.bfloat16),
            )
        )
    in_maps = []
    for c in range(8):
        m = dict(per_batch[c // 2])
        m.update(per_half[c % 2])
        in_maps.append(m)
    return in_maps


_NC_CACHE = {}


def _get_nc(dt_name="float32r", n_reps=1, rep_stage="both"):
    key = (dt_name, n_reps, rep_stage)
    if key not in _NC_CACHE:
        _NC_CACHE[key] = build_nc(
            DT=getattr(mybir.dt, dt_name), n_reps=n_reps, rep_stage=rep_stage
        )
    return _NC_CACHE[key]


# ---------------------------------------------------------------------------
# Cached PJRT runner.  Mirrors run_bass_kernel_spmd's axon redirect
# (bass2jax.run_bass_via_pjrt) but builds the jitted shard_map once per
# (dt_name, n_execs) so repeat kernel() calls skip re-tracing, and supports
# chaining n_execs sequential NEFF executions inside one program so test.py
# can measure per-execution hardware time as a slope (dispatch overhead
# cancels).
# ---------------------------------------------------------------------------
_RUNNER_CACHE = {}


def _get_runner(dt_name="float32r", n_reps=1, rep_stage="both"):
    key = (dt_name, n_reps, rep_stage)
    if key in _RUNNER_CACHE:
        return _RUNNER_CACHE[key]

    import jax
    from jax.sharding import Mesh, PartitionSpec
    from jax.experimental.shard_map import shard_map
    from concourse import bass2jax
    from concourse.bass2jax import _bass_exec_p

    bass2jax.install_neuronx_cc_hook()
    nc = _get_nc(dt_name, n_reps, rep_stage)
    partition_name = nc.partition_id_tensor.name if nc.partition_id_tensor else None

    in_names = []
    out_names = []
    out_avals = []
    for alloc in nc.m.functions[0].allocations:
        if not isinstance(alloc, mybir.MemoryLocationSet):
            continue
        name = alloc.memorylocations[0].name
        if alloc.kind == "ExternalInput":
            if name != partition_name:
                in_names.append(name)
        elif alloc.kind == "ExternalOutput":
            out_names.append(name)
            out_avals.append(
                jax.core.ShapedArray(tuple(alloc.tensor_shape), mybir.dt.np(alloc.dtype))
            )
    n_params = len(in_names)
    n_outs = len(out_avals)
    all_in_names = tuple(in_names + out_names)
    if partition_name is not None:
        all_in_names = all_in_names + (partition_name,)

    def _body(*args):
        params = list(args[:n_params])
        zeros = list(args[n_params:])
        pid = (
            [bass2jax.partition_id_tensor()] if partition_name is not None else []
        )
        outs = _bass_exec_p.bind(
            *params,
            *zeros,
            *pid,
            out_avals=tuple(out_avals),
            in_names=all_in_names,
            out_names=tuple(out_names),
            lowering_input_output_aliases=(),
            sim_require_finite=True,
            sim_require_nnan=True,
            nc=nc,
        )
        return tuple(outs)

    devices = jax.devices()[:8]
    mesh = Mesh(np.asarray(devices), ("core",))
    in_specs = (PartitionSpec("core"),) * (n_params + n_outs)
    out_specs = (PartitionSpec("core"),) * n_outs
    fn = jax.jit(
        shard_map(_body, mesh=mesh, in_specs=in_specs, out_specs=out_specs,
                  check_rep=False),
        keep_unused=True,
    )
    runner = (fn, in_names, out_names, out_avals)
    _RUNNER_CACHE[key] = runner
    return runner


def _concat_inputs(in_maps, in_names, out_avals, out_names):
    args = []
    for name in in_names:
        args.append(np.concatenate([np.asarray(m[name]) for m in in_maps], axis=0))
    for i, name in enumerate(out_names):
        z = out_avals[i]
        args.append(np.zeros((8 * z.shape[0], *z.shape[1:]), z.dtype))
    return args


def run(inputs, dt_name="float32r"):
    """Returns (full_output [B,S,D] f32, per-core outp list)."""
    fn, in_names, out_names, out_avals = _get_runner(dt_name, 1)
    in_maps = _prep_in_maps(**inputs)
    args = _concat_inputs(in_maps, in_names, out_avals, out_names)
    out_arrs = fn(*args)
    i = out_names.index("outp")
    per_core = np.asarray(out_arrs[i]).reshape(8, S, D)
    bo = np.asarray(inputs["bo"], dtype=np.float32)
    out = np.empty((B, S, D), dtype=np.float32)
    for b in range(B):
        out[b] = per_core[2 * b] + per_core[2 * b + 1] + bo
    return out, per_core


def bench(inputs, dt_name="float32r", n_reps=1, iters=6, rep_stage="both"):
    """Time the NEFF whose body repeats n_reps times on-device."""
    import time as _time
    import jax
    fn, in_names, out_names, out_avals = _get_runner(dt_name, n_reps, rep_stage)
    in_maps = _prep_in_maps(**inputs)
    args = _concat_inputs(in_maps, in_names, out_avals, out_names)
    dargs = [jax.device_put(a) for a in args]
    times = []
    for _ in range(iters):
        t0 = _time.perf_counter()
        outs = fn(*dargs)
        jax.block_until_ready(outs)
        times.append(_time.perf_counter() - t0)
    return times


def kernel(**inputs):
    inputs = {k: np.asarray(v) for k, v in inputs.items()}
    out, _ = run(inputs)
    return out
